# revision 45
# baseline (speedup 1.0000x reference)
"""CFConv (SchNet continuous-filter convolution) on 8 TRN2 NeuronCores.

Reference computation:
    f    = x @ W_in                       # (20000, 128)
    f_j  = f[idx_j]                       # (640000, 128) gather
    wf   = w_ij * f_j                     # elementwise
    conv = segment_sum(wf, seg_i)         # (20000, 128), seg_i sorted
    out  = conv @ W_out + b_out

Distribution: seg_i is sorted, so atoms are split into 8 contiguous
ranges of 2560 (padded to 20480); each core gets the edges targeting its
atom range.  No collectives needed — each core owns its output rows.

Per-core device pipeline (all matmuls bf16, f32 PSUM accumulate):
  Phase A: f = x @ W_in computed locally (replicated), written to an
           internal HBM table (bf16 rows).
  Phase B: edges processed in groups of 128 (one group = one matmul
           contraction).  Groups are host-packed per 128-atom window with
           a fixed per-window group count K_FIX (padding with zero
           edges), so the graph is identical on all cores (SPMD).
    - w_ij group tiles DMA'd from HBM (host-reordered, bf16)
    - f_j rows fetched with gpsimd.dma_gather (MoE gather primitive)
    - wf = w * f_j on VectorE
    - segment-sum via TensorE: psum[fm, atom_window] += wf_g^T @ S_g
      where S_g is the host-built 0/1 edge->atom one-hot matrix
    - out^T = W_out^T @ conv^T (TensorE), bias via ScalarE, transposed
      back per 128x128 tile on TensorE, DMA'd to the output shard.
"""

import numpy as np
import ml_dtypes

import concourse.bacc as bacc
import concourse.bass as bass
import concourse.mybir as mybir
import concourse.tile as tile
from concourse.bass_utils import run_bass_kernel_spmd

BF16 = ml_dtypes.bfloat16

N_ATOMS = 20000
N_EDGES = 640000
F = 128
N_CORES = 8
A_CORE = 2560                 # padded atoms per core
A_PAD = A_CORE * N_CORES      # 20480
CHUNK = 512                   # atoms per PSUM chunk (one bank)
N_CH = A_CORE // CHUNK        # 5
WIN = 128                     # atoms per window (matmul N dim)
WIN_PER_CORE = A_CORE // WIN  # 20
N_WIN = A_PAD // WIN          # 160

TRACE = False                 # set True (with ntff shim) for profiling
_BUILD_CACHE: dict = {}


def _build(k_lo: int, k_hi: int):
    """Build the SPMD Bass graph for given per-window group counts.

    Each 128-atom window's edges are split into a lo half (f rows
    [0, A_PAD/2)) and a hi half, each padded to k_lo/k_hi groups of 128;
    the two dma_gathers per window depend only on their half of the f
    table, so phase B overlaps the tail of phase A.
    """
    key = (k_lo, k_hi)
    if key in _BUILD_CACHE:
        return _BUILD_CACHE[key]

    k_fix = k_lo + k_hi
    G = WIN_PER_CORE * k_fix      # groups per core
    E = G * 128                   # padded edges per core
    H = A_PAD // 2
    bf = mybir.dt.bfloat16
    f32 = mybir.dt.float32

    nc = bacc.Bacc("TRN2", target_bir_lowering=False, debug=False,
                   num_swdge_queues=4, num_devices=N_CORES)
    xT_e = nc.dram_tensor("xT", [128, A_PAD], bf, kind="ExternalInput")
    w_in_e = nc.dram_tensor("w_in", [128, 128], bf, kind="ExternalInput")
    w_out_e = nc.dram_tensor("w_out", [128, 128], bf, kind="ExternalInput")
    b_e = nc.dram_tensor("b_out", [128, 1], f32, kind="ExternalInput")
    id_e = nc.dram_tensor("ident", [128, 128], bf, kind="ExternalInput")
    w_ed_e = nc.dram_tensor("w_ed", [128, G, F], bf, kind="ExternalInput")
    s_ed_e = nc.dram_tensor("s_ed", [128, G, WIN], mybir.dt.float8e4,
                            kind="ExternalInput")
    idx_e = nc.dram_tensor("idxw", [128, E // 16], mybir.dt.int16,
                           kind="ExternalInput")
    out_e = nc.dram_tensor("out", [A_CORE, F], f32, kind="ExternalOutput")

    with tile.TileContext(nc) as tc:
        with (
            tc.tile_pool(name="dram", bufs=1, space="DRAM") as dpool,
            tc.tile_pool(name="const", bufs=1) as cpool,
            tc.tile_pool(name="xqp", bufs=1) as xqpool,
            tc.tile_pool(name="pha", bufs=4) as apool,
            tc.tile_pool(name="psA", bufs=2, space="PSUM") as psA,
            tc.tile_pool(name="phb", bufs=3) as bpool,
            tc.tile_pool(name="wfp", bufs=4) as wfpool,
            tc.tile_pool(name="fjp", bufs=5) as fjpool,
            tc.tile_pool(name="psC", bufs=2, space="PSUM") as pscp,
            tc.tile_pool(name="ps2", bufs=2, space="PSUM") as ps2p,
            tc.tile_pool(name="ps3", bufs=2, space="PSUM") as ps3p,
        ):
            f_lo_hbm = dpool.tile([H, F], bf)
            f_hi_hbm = dpool.tile([H, F], bf)

            w_in_t = cpool.tile([128, 128], bf)
            nc.sync.dma_start(w_in_t[:], w_in_e[:])
            w_out_t = cpool.tile([128, 128], bf)
            nc.sync.dma_start(w_out_t[:], w_out_e[:])
            b_t = cpool.tile([128, 1], f32)
            nc.sync.dma_start(b_t[:], b_e[:])
            id_t = cpool.tile([128, 128], bf)
            nc.sync.dma_start(id_t[:], id_e[:])
            idx_t = cpool.tile([128, E // 16], mybir.dt.int16)
            nc.scalar.dma_start(idx_t[:], idx_e[:])

            # ---------------- Phase A: f table ----------------
            if True:
                QW = A_PAD // 4
                xq = []
                for x4 in range(4):
                    xq_t = xqpool.tile([128, QW], bf, tag=f"xq{x4}")
                    nc.sync.dma_start(
                        xq_t[:], xT_e[:, x4 * QW:(x4 + 1) * QW])
                    xq.append(xq_t)
                n_t = A_PAD // 128  # 160 atom tiles
                for t4 in range(n_t // 4):    # 4 matmuls per psum bank
                    ps = psA.tile([128, 4, 128], f32)
                    for q in range(4):
                        t = t4 * 4 + q
                        nc.tensor.matmul(
                            ps[:, q, :],
                            xq[t // 40][:, (t % 40) * 128:(t % 40 + 1) * 128],
                            w_in_t[:],
                            start=True, stop=True,
                        )
                    j = t4 % 2
                    if j == 0:
                        f_sb = apool.tile([128, 8, F], bf, tag="fsb")
                    nc.vector.tensor_copy(f_sb[:, j * 4:(j + 1) * 4, :], ps[:])
                    if j == 1:
                        a0 = (t4 - 1) * 512
                        tgt = f_lo_hbm if a0 < H else f_hi_hbm
                        a0 = a0 % H
                        dst = tgt[a0:a0 + 1024, :].rearrange(
                            "(j p) f -> p j f", p=128)
                        nc.sync.dma_start(dst, f_sb[:])

            # ---------------- Phase B: edges ----------------
            if True:
                psc = None
                for wk in range(WIN_PER_CORE):
                    ch = wk // 4
                    col = WIN * (wk % 4)

                    w_t = bpool.tile([128, k_fix, F], bf, tag="w")
                    nc.scalar.dma_start(
                        w_t[:], w_ed_e[:, wk * k_fix:(wk + 1) * k_fix, :])
                    s_t = bpool.tile([128, k_fix, WIN], mybir.dt.float8e4,
                                     tag="s")
                    nc.scalar.dma_start(
                        s_t[:], s_ed_e[:, wk * k_fix:(wk + 1) * k_fix, :])
                    base8 = wk * k_fix * 8
                    fj_lo = fjpool.tile([128, k_lo, F], bf, tag="fjlo")
                    nc.gpsimd.dma_gather(
                        fj_lo[:], f_lo_hbm[:, :],
                        idx_t[:, base8:base8 + k_lo * 8],
                        num_idxs=k_lo * 128,
                        num_idxs_reg=k_lo * 128,
                        elem_size=F,
                        single_packet=False,
                        queue_num=(2 * wk) % 4,
                    )
                    fj_hi = fjpool.tile([128, k_hi, F], bf, tag="fjhi")
                    nc.gpsimd.dma_gather(
                        fj_hi[:], f_hi_hbm[:, :],
                        idx_t[:, base8 + k_lo * 8:base8 + k_fix * 8],
                        num_idxs=k_hi * 128,
                        num_idxs_reg=k_hi * 128,
                        elem_size=F,
                        single_packet=False,
                        queue_num=(2 * wk + 1) % 4,
                    )
                    wf_lo = wfpool.tile([128, k_lo, F], bf, tag="wflo")
                    nc.vector.tensor_tensor(
                        wf_lo[:], w_t[:, 0:k_lo, :], fj_lo[:],
                        mybir.AluOpType.mult)
                    wf_hi = wfpool.tile([128, k_hi, F], bf, tag="wfhi")
                    nc.vector.tensor_tensor(
                        wf_hi[:], w_t[:, k_lo:k_fix, :], fj_hi[:],
                        mybir.AluOpType.mult)

                    if wk % 4 == 0:
                        psc = pscp.tile([128, CHUNK], f32)
                    for g in range(k_fix):
                        lhsT = (wf_lo[:, g, :] if g < k_lo
                                else wf_hi[:, g - k_lo, :])
                        nc.tensor.matmul(
                            psc[:, col:col + WIN],
                            lhsT,
                            s_t[:, g, :],
                            start=(g == 0), stop=(g == k_fix - 1),
                        )

                    if wk % 4 == 3:
                        convT = bpool.tile([128, CHUNK], bf, tag="convT")
                        nc.vector.tensor_copy(convT[:], psc[:])
                        ps2 = ps2p.tile([128, CHUNK], f32)
                        nc.tensor.matmul(ps2[:], w_out_t[:], convT[:],
                                         start=True, stop=True)
                        outT = bpool.tile([128, CHUNK], bf, tag="outT")
                        nc.scalar.activation(
                            outT[:], ps2[:],
                            mybir.ActivationFunctionType.Identity,
                            bias=b_t[:],
                        )
                        outf = bpool.tile([128, 4, F], f32, tag="outf")
                        for t in range(4):
                            ps3 = ps3p.tile([128, 128], bf)
                            nc.tensor.transpose(
                                ps3[:], outT[:, t * 128:(t + 1) * 128],
                                id_t[:])
                            nc.vector.tensor_copy(outf[:, t, :], ps3[:])
                        dst = out_e[ch * CHUNK:(ch + 1) * CHUNK, :].rearrange(
                            "(t p) f -> p t f", p=128)
                        nc.sync.dma_start(dst, outf[:])

    nc.compile()
    _BUILD_CACHE[key] = nc
    return nc


def _prep(x, w_ij, seg_i, idx_j, W_in, W_out, b_out):
    """Host-side sharding: reorder/pad edges, build S one-hots, wrap idxs."""
    x = np.asarray(x, dtype=np.float32)
    w_ij = np.asarray(w_ij, dtype=np.float32)
    seg = np.asarray(seg_i).astype(np.int64)
    idxj = np.asarray(idx_j).astype(np.int64)
    if not np.all(np.diff(seg) >= 0):
        order = np.argsort(seg, kind="stable")
        seg, idxj, w_ij = seg[order], idxj[order], w_ij[order]

    bounds = np.searchsorted(seg, np.arange(N_WIN + 1) * WIN)
    Hs = A_PAD // 2

    # per-window lo/hi split (f-table halves)
    lo_ids, hi_ids, lo_v, hi_v = [], [], [], []
    n_lo = np.zeros(N_WIN, np.int64)
    n_hi = np.zeros(N_WIN, np.int64)
    for k in range(N_WIN):
        b0, b1 = bounds[k], bounds[k + 1]
        ids = np.arange(b0, b1)
        v = idxj[b0:b1]
        m = v < Hs
        lo_ids.append(ids[m])
        hi_ids.append(ids[~m])
        lo_v.append(v[m].astype(np.int16))
        hi_v.append((v[~m] - Hs).astype(np.int16))
        n_lo[k] = m.sum()
        n_hi[k] = (~m).sum()
    k_lo = max(1, int(np.ceil(n_lo.max() / 128)))
    k_hi = max(1, int(np.ceil(n_hi.max() / 128)))
    k_fix = k_lo + k_hi
    e_win = k_fix * 128
    g_core = WIN_PER_CORE * k_fix
    e_pad = g_core * 128

    # padded edge-id + gather-idx matrices in lo|hi order
    eidx = np.zeros((N_WIN, e_win), np.int64)
    valid = np.zeros((N_WIN, e_win), bool)
    gidx = np.zeros((N_WIN, e_win), np.int16)
    for k in range(N_WIN):
        a, b = n_lo[k], n_hi[k]
        eidx[k, :a] = lo_ids[k]
        valid[k, :a] = True
        gidx[k, :a] = lo_v[k]
        off = k_lo * 128
        eidx[k, off:off + b] = hi_ids[k]
        valid[k, off:off + b] = True
        gidx[k, off:off + b] = hi_v[k]

    w_bf = w_ij.astype(BF16)
    seg16 = seg.astype(np.int64)

    xT = np.zeros((128, A_PAD), BF16)
    xT[:, :N_ATOMS] = np.ascontiguousarray(x.T).astype(BF16)
    shared = {
        "xT": xT,
        "w_in": np.asarray(W_in, np.float32).astype(BF16),
        "w_out": np.asarray(W_out, np.float32).astype(BF16),
        "b_out": np.asarray(b_out, np.float32).reshape(128, 1).copy(),
        "ident": np.eye(128, dtype=BF16),
    }

    in_maps = []
    for c in range(N_CORES):
        sl = slice(c * WIN_PER_CORE, (c + 1) * WIN_PER_CORE)
        ei = eidx[sl].reshape(-1)
        va = valid[sl].reshape(-1)

        w_rows = np.zeros((e_pad, F), BF16)
        w_rows[va] = w_bf[ei[va]]
        w_ed = np.ascontiguousarray(
            w_rows.reshape(g_core, 128, F).transpose(1, 0, 2))

        wb = (np.arange(c * WIN_PER_CORE, (c + 1) * WIN_PER_CORE)
              * WIN).repeat(e_win)
        rel = seg16[ei] - wb
        s_rows = np.zeros((e_pad, WIN), ml_dtypes.float8_e4m3)
        vrows = np.nonzero(va)[0]
        s_rows[vrows, rel[vrows]] = 1
        s_ed = np.ascontiguousarray(
            s_rows.reshape(g_core, 128, WIN).transpose(1, 0, 2))

        # wrapped idx layout, one wrap per gather call (lo and hi per window)
        gi = gidx[sl]                              # [20, e_win]
        blocks = []
        for wkk in range(WIN_PER_CORE):
            lo_blk = gi[wkk, :k_lo * 128].reshape(-1, 16).T    # [16, k_lo*8]
            hi_blk = gi[wkk, k_lo * 128:].reshape(-1, 16).T    # [16, k_hi*8]
            blocks.append(lo_blk)
            blocks.append(hi_blk)
        idxw = np.ascontiguousarray(
            np.tile(np.concatenate(blocks, axis=1), (8, 1)))

        m = dict(shared)
        m["w_ed"] = w_ed
        m["s_ed"] = s_ed
        m["idxw"] = idxw
        in_maps.append(m)
    return k_lo, k_hi, in_maps


def kernel(x, w_ij, seg_i, idx_j, seg_i_sum, W_in, W_out, b_out):
    k_lo, k_hi, in_maps = _prep(x, w_ij, seg_i, idx_j, W_in, W_out, b_out)
    nc = _build(k_lo, k_hi)
    res = run_bass_kernel_spmd(nc, in_maps, core_ids=list(range(N_CORES)),
                               trace=TRACE)
    kernel.last_result = res
    out = np.concatenate(
        [np.asarray(res.results[c]["out"]) for c in range(N_CORES)], axis=0)
    return np.ascontiguousarray(out[:N_ATOMS]).astype(np.float32)


# revision 46
# speedup vs baseline: 1.3250x; 1.3250x over previous
"""CFConv (SchNet continuous-filter convolution) on 8 TRN2 NeuronCores.

Reference computation:
    f    = x @ W_in                       # (20000, 128)
    f_j  = f[idx_j]                       # (640000, 128) gather
    wf   = w_ij * f_j                     # elementwise
    conv = segment_sum(wf, seg_i)         # (20000, 128), seg_i sorted
    out  = conv @ W_out + b_out

Distribution: seg_i is sorted, so atoms are split into 8 contiguous
ranges of 2560 (padded to 20480); each core gets the edges targeting its
atom range.  No collectives needed — each core owns its output rows.

Per-core device pipeline (all matmuls bf16, f32 PSUM accumulate):
  Phase A: f = x @ W_in computed locally (replicated), written to an
           internal HBM table (bf16 rows).
  Phase B: edges processed in groups of 128 (one group = one matmul
           contraction).  Groups are host-packed per 128-atom window with
           a fixed per-window group count K_FIX (padding with zero
           edges), so the graph is identical on all cores (SPMD).
    - w_ij group tiles DMA'd from HBM (host-reordered, bf16)
    - f_j rows fetched with gpsimd.dma_gather (MoE gather primitive)
    - wf = w * f_j on VectorE
    - segment-sum via TensorE: psum[fm, atom_window] += wf_g^T @ S_g
      where S_g is the host-built 0/1 edge->atom one-hot matrix
    - out^T = W_out^T @ conv^T (TensorE), bias via ScalarE, transposed
      back per 128x128 tile on TensorE, DMA'd to the output shard.
"""

import numpy as np
import ml_dtypes

import concourse.bacc as bacc
import concourse.bass as bass
import concourse.mybir as mybir
import concourse.tile as tile
from concourse.bass_utils import run_bass_kernel_spmd

BF16 = ml_dtypes.bfloat16

N_ATOMS = 20000
N_EDGES = 640000
F = 128
N_CORES = 8
A_CORE = 2560                 # padded atoms per core
A_PAD = A_CORE * N_CORES      # 20480
CHUNK = 512                   # atoms per PSUM chunk (one bank)
N_CH = A_CORE // CHUNK        # 5
WIN = 128                     # atoms per window (matmul N dim)
WIN_PER_CORE = A_CORE // WIN  # 20
N_WIN = A_PAD // WIN          # 160

TRACE = False                 # set True (with ntff shim) for profiling
_BUILD_CACHE: dict = {}


def _build(k_lo: int, k_hi: int):
    """Build the SPMD Bass graph for given per-window group counts.

    Each 128-atom window's edges are split into a lo half (f rows
    [0, A_PAD/2)) and a hi half, each padded to k_lo/k_hi groups of 128;
    the two dma_gathers per window depend only on their half of the f
    table, so phase B overlaps the tail of phase A.
    """
    key = (k_lo, k_hi)
    if key in _BUILD_CACHE:
        return _BUILD_CACHE[key]

    k_fix = k_lo + k_hi
    G = WIN_PER_CORE * k_fix      # groups per core
    E = G * 128                   # padded edges per core
    H = A_PAD // 2
    bf = mybir.dt.bfloat16
    f32 = mybir.dt.float32

    nc = bacc.Bacc("TRN2", target_bir_lowering=False, debug=False,
                   num_swdge_queues=4, num_devices=N_CORES)
    xT_e = nc.dram_tensor("xT", [128, A_PAD], bf, kind="ExternalInput")
    w_in_e = nc.dram_tensor("w_in", [128, 128], bf, kind="ExternalInput")
    w_out_e = nc.dram_tensor("w_out", [128, 128], bf, kind="ExternalInput")
    b_e = nc.dram_tensor("b_out", [128, 1], f32, kind="ExternalInput")
    id_e = nc.dram_tensor("ident", [128, 128], bf, kind="ExternalInput")
    w_ed_e = nc.dram_tensor("w_ed", [128, G, F], bf, kind="ExternalInput")
    s_ed_e = nc.dram_tensor("s_ed", [128, G, WIN], mybir.dt.float8e4,
                            kind="ExternalInput")
    idx_e = nc.dram_tensor("idxw", [128, E // 16], mybir.dt.int16,
                           kind="ExternalInput")
    out_e = nc.dram_tensor("out", [A_CORE, F], f32, kind="ExternalOutput")

    with tile.TileContext(nc) as tc:
        with (
            tc.tile_pool(name="dram", bufs=1, space="DRAM") as dpool,
            tc.tile_pool(name="const", bufs=1) as cpool,
        ):
            f_lo_hbm = dpool.tile([H, F], bf)
            f_hi_hbm = dpool.tile([H, F], bf)

            w_in_t = cpool.tile([128, 128], bf)
            nc.sync.dma_start(w_in_t[:], w_in_e[:])
            w_out_t = cpool.tile([128, 128], bf)
            nc.sync.dma_start(w_out_t[:], w_out_e[:])
            b_t = cpool.tile([128, 1], f32)
            nc.sync.dma_start(b_t[:], b_e[:])
            id_t = cpool.tile([128, 128], bf)
            nc.sync.dma_start(id_t[:], id_e[:])
            idx_t = cpool.tile([128, E // 16], mybir.dt.int16)
            nc.scalar.dma_start(idx_t[:], idx_e[:])

            # ---------------- Phase A: f table ----------------
            with (
                tc.tile_pool(name="pha", bufs=4) as apool,
                tc.tile_pool(name="psA", bufs=4, space="PSUM") as psA,
            ):
                xT_t = apool.tile([128, A_PAD], bf)
                for x4 in range(4):
                    nc.sync.dma_start(
                        xT_t[:, x4 * (A_PAD // 4):(x4 + 1) * (A_PAD // 4)],
                        xT_e[:, x4 * (A_PAD // 4):(x4 + 1) * (A_PAD // 4)])
                n_t = A_PAD // 128  # 160 atom tiles
                for t4 in range(n_t // 4):    # 4 matmuls per psum bank
                    ps = psA.tile([128, 4, 128], f32)
                    for q in range(4):
                        t = t4 * 4 + q
                        nc.tensor.matmul(
                            ps[:, q, :],
                            xT_t[:, t * 128:(t + 1) * 128],
                            w_in_t[:],
                            start=True, stop=True,
                        )
                    j = t4 % 2
                    if j == 0:
                        f_sb = apool.tile([128, 8, F], bf, tag="fsb")
                    nc.vector.tensor_copy(f_sb[:, j * 4:(j + 1) * 4, :], ps[:])
                    if j == 1:
                        a0 = (t4 - 1) * 512
                        tgt = f_lo_hbm if a0 < H else f_hi_hbm
                        a0 = a0 % H
                        dst = tgt[a0:a0 + 1024, :].rearrange(
                            "(j p) f -> p j f", p=128)
                        nc.sync.dma_start(dst, f_sb[:])

            # ---------------- Phase B: edges ----------------
            with (
                tc.tile_pool(name="phb", bufs=3) as bpool,
                tc.tile_pool(name="fjp", bufs=8) as fjpool,
                tc.tile_pool(name="psC", bufs=2, space="PSUM") as pscp,
                tc.tile_pool(name="ps2", bufs=2, space="PSUM") as ps2p,
                tc.tile_pool(name="ps3", bufs=2, space="PSUM") as ps3p,
            ):
                psc = None
                for wk in range(WIN_PER_CORE):
                    ch = wk // 4
                    col = WIN * (wk % 4)

                    w_t = bpool.tile([128, k_fix, F], bf, tag="w")
                    nc.scalar.dma_start(
                        w_t[:], w_ed_e[:, wk * k_fix:(wk + 1) * k_fix, :])
                    s_t = bpool.tile([128, k_fix, WIN], mybir.dt.float8e4,
                                     tag="s")
                    nc.scalar.dma_start(
                        s_t[:], s_ed_e[:, wk * k_fix:(wk + 1) * k_fix, :])
                    base8 = wk * k_fix * 8
                    fj_t = fjpool.tile([128, k_fix, F], bf, tag="fj")
                    nc.gpsimd.dma_gather(
                        fj_t[:, 0:k_lo, :], f_lo_hbm[:, :],
                        idx_t[:, base8:base8 + k_lo * 8],
                        num_idxs=k_lo * 128,
                        num_idxs_reg=k_lo * 128,
                        elem_size=F,
                        single_packet=False,
                        queue_num=(2 * wk) % 4,
                    )
                    nc.gpsimd.dma_gather(
                        fj_t[:, k_lo:k_fix, :], f_hi_hbm[:, :],
                        idx_t[:, base8 + k_lo * 8:base8 + k_fix * 8],
                        num_idxs=k_hi * 128,
                        num_idxs_reg=k_hi * 128,
                        elem_size=F,
                        single_packet=False,
                        queue_num=(2 * wk + 1) % 4,
                    )
                    wf_t = bpool.tile([128, k_fix, F], bf, tag="wf")
                    nc.vector.tensor_tensor(
                        wf_t[:], w_t[:], fj_t[:], mybir.AluOpType.mult)

                    if wk % 4 == 0:
                        psc = pscp.tile([128, CHUNK], f32)
                    for g in range(k_fix):
                        nc.tensor.matmul(
                            psc[:, col:col + WIN],
                            wf_t[:, g, :],
                            s_t[:, g, :],
                            start=(g == 0), stop=(g == k_fix - 1),
                        )

                    if wk % 4 == 3:
                        convT = bpool.tile([128, CHUNK], bf, tag="convT")
                        nc.vector.tensor_copy(convT[:], psc[:])
                        ps2 = ps2p.tile([128, CHUNK], f32)
                        nc.tensor.matmul(ps2[:], w_out_t[:], convT[:],
                                         start=True, stop=True)
                        outT = bpool.tile([128, CHUNK], bf, tag="outT")
                        nc.scalar.activation(
                            outT[:], ps2[:],
                            mybir.ActivationFunctionType.Identity,
                            bias=b_t[:],
                        )
                        outf = bpool.tile([128, 4, F], f32, tag="outf")
                        for t in range(4):
                            ps3 = ps3p.tile([128, 128], bf)
                            nc.tensor.transpose(
                                ps3[:], outT[:, t * 128:(t + 1) * 128],
                                id_t[:])
                            nc.vector.tensor_copy(outf[:, t, :], ps3[:])
                        dst = out_e[ch * CHUNK:(ch + 1) * CHUNK, :].rearrange(
                            "(t p) f -> p t f", p=128)
                        nc.sync.dma_start(dst, outf[:])

    nc.compile()
    _BUILD_CACHE[key] = nc
    return nc


def _prep(x, w_ij, seg_i, idx_j, W_in, W_out, b_out):
    """Host-side sharding: reorder/pad edges, build S one-hots, wrap idxs."""
    x = np.asarray(x, dtype=np.float32)
    w_ij = np.asarray(w_ij, dtype=np.float32)
    seg = np.asarray(seg_i).astype(np.int64)
    idxj = np.asarray(idx_j).astype(np.int64)
    if not np.all(np.diff(seg) >= 0):
        order = np.argsort(seg, kind="stable")
        seg, idxj, w_ij = seg[order], idxj[order], w_ij[order]

    bounds = np.searchsorted(seg, np.arange(N_WIN + 1) * WIN)
    Hs = A_PAD // 2

    # per-window lo/hi split (f-table halves)
    lo_ids, hi_ids, lo_v, hi_v = [], [], [], []
    n_lo = np.zeros(N_WIN, np.int64)
    n_hi = np.zeros(N_WIN, np.int64)
    for k in range(N_WIN):
        b0, b1 = bounds[k], bounds[k + 1]
        ids = np.arange(b0, b1)
        v = idxj[b0:b1]
        m = v < Hs
        lo_ids.append(ids[m])
        hi_ids.append(ids[~m])
        lo_v.append(v[m].astype(np.int16))
        hi_v.append((v[~m] - Hs).astype(np.int16))
        n_lo[k] = m.sum()
        n_hi[k] = (~m).sum()
    k_lo = max(1, int(np.ceil(n_lo.max() / 128)))
    k_hi = max(1, int(np.ceil(n_hi.max() / 128)))
    k_fix = k_lo + k_hi
    e_win = k_fix * 128
    g_core = WIN_PER_CORE * k_fix
    e_pad = g_core * 128

    # padded edge-id + gather-idx matrices in lo|hi order
    eidx = np.zeros((N_WIN, e_win), np.int64)
    valid = np.zeros((N_WIN, e_win), bool)
    gidx = np.zeros((N_WIN, e_win), np.int16)
    for k in range(N_WIN):
        a, b = n_lo[k], n_hi[k]
        eidx[k, :a] = lo_ids[k]
        valid[k, :a] = True
        gidx[k, :a] = lo_v[k]
        off = k_lo * 128
        eidx[k, off:off + b] = hi_ids[k]
        valid[k, off:off + b] = True
        gidx[k, off:off + b] = hi_v[k]

    w_bf = w_ij.astype(BF16)
    seg16 = seg.astype(np.int64)

    xT = np.zeros((128, A_PAD), BF16)
    xT[:, :N_ATOMS] = np.ascontiguousarray(x.T).astype(BF16)
    shared = {
        "xT": xT,
        "w_in": np.asarray(W_in, np.float32).astype(BF16),
        "w_out": np.asarray(W_out, np.float32).astype(BF16),
        "b_out": np.asarray(b_out, np.float32).reshape(128, 1).copy(),
        "ident": np.eye(128, dtype=BF16),
    }

    in_maps = []
    for c in range(N_CORES):
        sl = slice(c * WIN_PER_CORE, (c + 1) * WIN_PER_CORE)
        ei = eidx[sl].reshape(-1)
        va = valid[sl].reshape(-1)

        w_rows = np.zeros((e_pad, F), BF16)
        w_rows[va] = w_bf[ei[va]]
        w_ed = np.ascontiguousarray(
            w_rows.reshape(g_core, 128, F).transpose(1, 0, 2))

        wb = (np.arange(c * WIN_PER_CORE, (c + 1) * WIN_PER_CORE)
              * WIN).repeat(e_win)
        rel = seg16[ei] - wb
        s_rows = np.zeros((e_pad, WIN), ml_dtypes.float8_e4m3)
        vrows = np.nonzero(va)[0]
        s_rows[vrows, rel[vrows]] = 1
        s_ed = np.ascontiguousarray(
            s_rows.reshape(g_core, 128, WIN).transpose(1, 0, 2))

        # wrapped idx layout, one wrap per gather call (lo and hi per window)
        gi = gidx[sl]                              # [20, e_win]
        blocks = []
        for wkk in range(WIN_PER_CORE):
            lo_blk = gi[wkk, :k_lo * 128].reshape(-1, 16).T    # [16, k_lo*8]
            hi_blk = gi[wkk, k_lo * 128:].reshape(-1, 16).T    # [16, k_hi*8]
            blocks.append(lo_blk)
            blocks.append(hi_blk)
        idxw = np.ascontiguousarray(
            np.tile(np.concatenate(blocks, axis=1), (8, 1)))

        m = dict(shared)
        m["w_ed"] = w_ed
        m["s_ed"] = s_ed
        m["idxw"] = idxw
        in_maps.append(m)
    return k_lo, k_hi, in_maps


def kernel(x, w_ij, seg_i, idx_j, seg_i_sum, W_in, W_out, b_out):
    k_lo, k_hi, in_maps = _prep(x, w_ij, seg_i, idx_j, W_in, W_out, b_out)
    nc = _build(k_lo, k_hi)
    res = run_bass_kernel_spmd(nc, in_maps, core_ids=list(range(N_CORES)),
                               trace=TRACE)
    kernel.last_result = res
    out = np.concatenate(
        [np.asarray(res.results[c]["out"]) for c in range(N_CORES)], axis=0)
    return np.ascontiguousarray(out[:N_ATOMS]).astype(np.float32)
